# revision 1
# baseline (speedup 1.0000x reference)
"""Trainium2 Bass kernel for nn_DeepNoSAF (6-layer GENConv-style GNN).

Sharding: nodes partitioned across 8 cores by dst range; each core owns the
incoming edges of its nodes (host sorts/pads edges into per-window chunks of
128).  Node state h is replicated in HBM (bf16) for the per-edge gather
(indirect DMA); updated slices are exchanged per layer with an AllGather.
Per-channel segment softmax is computed with one-hot matmuls accumulating
num=sum(e*m), den=sum(e) in PSUM per 128-dst window (max-subtraction skipped;
the +1e-16 denominator keeps empty segments at 0).  Node phase runs F-major
(weights stationary); LayerNorm stats via ones-matmuls, broadcasts via K=1
matmuls; all transcendentals use one ACT table set {exp, ln, lrelu, square}:
sigmoid(x)=exp(-ln(1+exp(-x))), rsqrt(x)=exp(-0.5*ln(x)).
"""

import os
import sys

sys.path.insert(0, "/opt/trn_rl_repo")

import numpy as np
import ml_dtypes

# ---------------- problem constants (hardcoded per spec) ----------------
N = 100000
E = 625000
F = 128
L = 6
HID = 64
NTOT = 200000
TASKS = 112
LN_EPS = 1e-5
C = 8                      # cores
S_NODES = N // C           # 12500 owned nodes per core
NWIN = 100                 # windows per core
WIN = 128                  # dst slots per window
SLICE = NWIN * WIN         # 12800 node slots per core
NGRP = NWIN // 4           # 25 groups of 4 windows (512 node cols)

BF16 = ml_dtypes.bfloat16


# ---------------- host-side graph packing ----------------
NBK = 4


def _pack_graph(edge_index):
    src = np.asarray(edge_index[0], dtype=np.int64)
    dst = np.asarray(edge_index[1], dtype=np.int64)
    core_of = dst // S_NODES
    BANK = C * SLICE // NBK

    deg = np.bincount(dst, minlength=N)
    node_win = np.full(N, -1, np.int32)
    node_pos = np.full(N, -1, np.int32)
    loads = np.zeros((C, NWIN), np.int64)

    avg = int(deg.sum()) // (C * NWIN)
    base_cap = max(WIN, (avg // WIN) * WIN)
    n_hi = max(1, (NWIN * 3) // 10)
    targets = np.array([base_cap + WIN] * n_hi + [base_cap] * (NWIN - n_hi),
                       np.int64)
    NEG = np.iinfo(np.int64).min
    for c in range(C):
        lo = c * S_NODES
        nodes = lo + np.argsort(-deg[lo:lo + S_NODES], kind="stable")
        counts = np.zeros(NWIN, np.int32)
        ld = loads[c]
        for n in nodes:
            room = targets - ld
            room[counts >= WIN] = NEG
            w = int(np.argmax(room))
            node_win[n] = w
            node_pos[n] = counts[w]
            counts[w] += 1
            ld[w] += deg[n]

    perm = np.full((C, SLICE), -1, np.int64)
    alln = np.arange(N)
    slot_global = node_win[alln] * WIN + node_pos[alln]
    perm[(alln // S_NODES), slot_global] = alln
    hrow = (alln // S_NODES) * SLICE + slot_global

    ew = node_win[dst]
    ebk = (hrow[src] // BANK).astype(np.int64)
    cnt = np.zeros((C, NWIN, NBK), np.int64)
    np.add.at(cnt, (core_of, ew, ebk), 1)
    K = np.maximum(1, -(-cnt.max(axis=0) // WIN))

    wb_base = np.zeros((NWIN, NBK), np.int64)
    off = 0
    half_meta = []
    for wlo in range(0, NWIN, 2):
        h0 = off
        runs = []
        for b in range(NBK):
            for w in (wlo, wlo + 1):
                wb_base[w, b] = off
                runs.append((b, w, int(K[w, b]), off))
                off += int(K[w, b]) * WIN
        half_meta.append((h0, off, runs))
    nslot = int(off)
    totch = nslot // WIN

    order = np.lexsort((np.arange(E), ebk, ew, core_of))
    src_s, dst_s, core_s = src[order], dst[order], core_of[order]
    grp_key = core_s * (NWIN * NBK) + ew[order] * NBK + ebk[order]
    uniq, start_idx, cnts = np.unique(grp_key, return_index=True,
                                      return_counts=True)
    run = np.arange(E) - np.repeat(start_idx, cnts)
    slot_of_edge = wb_base[ew[order], ebk[order]] + run

    src_img = np.zeros((C, totch * WIN), np.int32)
    dloc_img = np.full((C, totch * WIN), -1.0, np.float32)
    eperm = np.full((C, nslot), -1, np.int64)
    i16 = np.zeros((C, nslot), np.int16)
    src_img[core_s, slot_of_edge] = hrow[src_s].astype(np.int32)
    i16.reshape(C, -1)[core_s, slot_of_edge] = (hrow[src_s] % BANK).astype(np.int16)
    dloc_img[core_s, slot_of_edge] = node_pos[dst_s].astype(np.float32)
    eperm[core_s, slot_of_edge] = order

    idx32_img = np.ascontiguousarray(
        src_img.reshape(C, totch, WIN).transpose(0, 2, 1))
    dl_img = np.ascontiguousarray(
        dloc_img.reshape(C, totch, WIN).transpose(0, 2, 1))
    idx16_img = np.zeros((C, 128, nslot // 16), np.int16)
    sl = np.arange(nslot)
    for k in range(8):
        idx16_img[:, sl % 16 + 16 * k, sl // 16] = i16

    return dict(K=K, half_meta=half_meta, nslot=nslot, totch=totch,
                perm=perm, idx32_img=idx32_img, idx16_img=idx16_img,
                dl_img=dl_img, eperm=eperm, BANK=BANK)


def _build_inputs(inputs, pk):
    x = np.asarray(inputs["x"], np.float32)
    node_index = np.asarray(inputs["node_index"]).astype(np.int64)
    edge_attr = np.asarray(inputs["edge_attr"], np.float32)
    table = np.asarray(inputs["node_features_table"], np.float32)

    perm, eperm = pk["perm"], pk["eperm"]
    nslot = pk["nslot"]

    w_enc = np.asarray(inputs["W_enc"], np.float32)
    b_enc = np.asarray(inputs["b_enc"], np.float32)
    w_ohe = np.asarray(inputs["W_ohe"], np.float32)
    b_ohe = np.asarray(inputs["b_ohe"], np.float32)
    w_edge = np.asarray(inputs["W_edge"], np.float32)
    b_edge = np.asarray(inputs["b_edge"], np.float32)

    # nf row order on device: [u (x@W_ohe+b_ohe) rows 0-7; tg rows 8-15; ones]
    wenc_aug = np.concatenate([w_enc[8:16], w_enc[0:8], b_enc[None, :]], 0)
    wx_aug = np.concatenate([w_ohe, b_ohe[None, :]], 0)            # [9,8]
    wedge_aug = np.concatenate([w_edge, b_edge[None, :]], 0)       # [9,128]

    gcnw = np.asarray(inputs["gcn_W"], np.float32).reshape(-1, F)
    w1 = np.asarray(inputs["learner_W1"], np.float32).reshape(-1, HID)
    w2 = np.asarray(inputs["learner_W2"], np.float32).reshape(-1, F)
    wpred = np.asarray(inputs["W_pred"], np.float32)

    pb = np.zeros((F, 26), np.float32)
    pb[:, 0:6] = np.asarray(inputs["gcn_b"], np.float32).T
    pb[:, 6:12] = np.asarray(inputs["ln_gamma"], np.float32).T
    pb[:, 12:18] = np.asarray(inputs["ln_beta"], np.float32).T
    pb[:, 18:25] = -np.asarray(inputs["learner_b2"], np.float32).T
    pb[:TASKS, 25] = np.asarray(inputs["b_pred"], np.float32)
    b1s = np.ascontiguousarray(np.asarray(inputs["learner_b1"], np.float32).T)

    iota = np.broadcast_to(np.arange(128, dtype=np.float32)[None, :],
                           (128, 128)).astype(BF16)
    ident = np.eye(128, dtype=np.float32)
    rowc = np.zeros((1, 256), np.float32)
    rowc[0, :128] = 1.0
    rowc[0, 128:] = -1.0 / 128.0
    colc = np.zeros((128, 4), np.float32)
    colc[:, 0] = 1.0
    colc[:, 1] = 1e-16
    colc[:, 2] = LN_EPS

    maps = []
    for c in range(C):
        pm = perm[c]
        valid = pm >= 0
        xs = np.zeros((SLICE, 8), np.float32)
        xs[valid] = x[pm[valid]]
        tg = np.zeros((SLICE, 8), np.float32)
        tg[valid] = table[node_index[pm[valid]]]
        xT9 = np.zeros((9, SLICE), np.float32)
        xT9[:8] = xs.T
        xT9[8] = 1.0
        tgT = np.ascontiguousarray(tg.T)

        ep = eperm[c]
        ev = ep >= 0
        ea = np.zeros((nslot, 8), np.float32)
        ea[ev] = edge_attr[ep[ev]]
        attrT = np.zeros((9, nslot), np.float32)
        attrT[:8] = ea.T
        attrT[8, :] = 1.0

        maps.append({
            "idx16": pk["idx16_img"][c], "dloc": pk["dl_img"][c],
            "attrT": attrT, "xT9": xT9, "tgT": tgT,
            "iota": np.ascontiguousarray(iota), "ident": ident,
            "rowc": rowc, "colc": colc,
            "wenc": wenc_aug, "wx": wx_aug, "wedge": wedge_aug,
            "gcnw": gcnw, "w1": w1, "w2": w2, "wpred": wpred,
            "pb": pb, "b1s": b1s,
        })
    return maps


DEBUG_DUMP = False


# ---------------- bass program ----------------
def _build_program(pk, tvals):
    import concourse.bass as bass
    import concourse.tile as tile
    from concourse import bacc, mybir

    dt = mybir.dt
    AF = mybir.ActivationFunctionType
    OP = mybir.AluOpType

    K, half_meta = pk["K"], pk["half_meta"]
    nslot, totch, BANK = pk["nslot"], pk["totch"], pk["BANK"]
    HROWS = C * SLICE

    nc = bacc.Bacc(num_devices=C)

    t_idx16 = nc.dram_tensor("idx16", [128, nslot // 16], dt.int16,
                             kind="ExternalInput")
    t_dloc = nc.dram_tensor("dloc", [128, totch], dt.float32, kind="ExternalInput")
    t_attr = nc.dram_tensor("attrT", [9, nslot], dt.float32, kind="ExternalInput")
    t_x = nc.dram_tensor("xT9", [9, SLICE], dt.float32, kind="ExternalInput")
    t_tg = nc.dram_tensor("tgT", [8, SLICE], dt.float32, kind="ExternalInput")
    t_iota = nc.dram_tensor("iota", [128, 128], dt.bfloat16, kind="ExternalInput")
    t_ident = nc.dram_tensor("ident", [128, 128], dt.float32, kind="ExternalInput")
    t_rowc = nc.dram_tensor("rowc", [1, 256], dt.float32, kind="ExternalInput")
    t_colc = nc.dram_tensor("colc", [128, 4], dt.float32, kind="ExternalInput")
    t_wenc = nc.dram_tensor("wenc", [17, 128], dt.float32, kind="ExternalInput")
    t_wx = nc.dram_tensor("wx", [9, 8], dt.float32, kind="ExternalInput")
    t_wedge = nc.dram_tensor("wedge", [9, 128], dt.float32, kind="ExternalInput")
    t_gcnw = nc.dram_tensor("gcnw", [6 * F, F], dt.float32, kind="ExternalInput")
    t_w1 = nc.dram_tensor("w1", [7 * F, HID], dt.float32, kind="ExternalInput")
    t_w2 = nc.dram_tensor("w2", [7 * HID, F], dt.float32, kind="ExternalInput")
    t_wpred = nc.dram_tensor("wpred", [F, TASKS], dt.float32, kind="ExternalInput")
    t_pb = nc.dram_tensor("pb", [128, 26], dt.float32, kind="ExternalInput")
    t_b1 = nc.dram_tensor("b1s", [HID, 7], dt.float32, kind="ExternalInput")
    t_out = nc.dram_tensor("out", [SLICE, TASKS], dt.float32, kind="ExternalOutput")
    t_dbg = None
    if DEBUG_DUMP:
        t_dbg = nc.dram_tensor("dbg", [128, 2 * SLICE], dt.float32,
                               kind="ExternalOutput")


    h_full = nc.dram_tensor("h_full", [HROWS, F], dt.bfloat16)
    h_stage = nc.dram_tensor("h_stage", [SLICE, F], dt.bfloat16)
    emb_dram = nc.dram_tensor("emb_dram", [128, totch, 128], dt.bfloat16)

    rg = [list(range(C))]
    # edge phase runs per half-group (2 windows) to bound SBUF tiles
    hmax = max(h1 - h0 for (h0, h1, _) in half_meta)

    with tile.TileContext(nc) as tc:
        with (
            tc.tile_pool(name="const", bufs=1) as cp,
            tc.tile_pool(name="state", bufs=1) as sp,
            tc.tile_pool(name="work", bufs=8) as wp,
            tc.tile_pool(name="work2", bufs=2) as wp2,
            tc.tile_pool(name="edge", bufs=2) as epool,
            tc.tile_pool(name="edge1", bufs=1) as ep1,
            tc.tile_pool(name="psum", bufs=1, space="PSUM") as pp,
            tc.tile_pool(name="psum2", bufs=1, space="PSUM") as pp2,
        ):
            def load_const(tt, shape, dtype):
                s = cp.tile(shape, dtype, tag=tt.name, name=tt.name + "_sb")
                nc.sync.dma_start(out=s[:], in_=tt[:])
                return s

            iota_sb = load_const(t_iota, [128, 128], dt.bfloat16)
            ident_sb = load_const(t_ident, [128, 128], dt.float32)
            rowc_sb = load_const(t_rowc, [1, 256], dt.float32)
            colc_sb = load_const(t_colc, [128, 4], dt.float32)
            wenc_sb = load_const(t_wenc, [17, 128], dt.float32)
            wx_sb = load_const(t_wx, [9, 8], dt.float32)
            wedge_sb = load_const(t_wedge, [9, 128], dt.float32)
            wpred_sb = load_const(t_wpred, [128, TASKS], dt.float32)
            pb_sb = load_const(t_pb, [128, 26], dt.float32)
            b1_sb = load_const(t_b1, [HID, 7], dt.float32)
            i16_sb = load_const(t_idx16, [128, nslot // 16], dt.int16)
            dloc_sb = load_const(t_dloc, [128, totch], dt.float32)

            gcnw_sb = cp.tile([128, 6 * 128], dt.float32)
            w1_sb = cp.tile([128, 7 * HID], dt.float32)
            w2_sb = cp.tile([HID, 7 * 128], dt.float32)
            for l in range(6):
                nc.sync.dma_start(out=gcnw_sb[:, l * 128:(l + 1) * 128],
                                  in_=t_gcnw[l * 128:(l + 1) * 128, :])
            for l in range(7):
                nc.sync.dma_start(out=w1_sb[:, l * HID:(l + 1) * HID],
                                  in_=t_w1[l * F:(l + 1) * F, :])
                nc.sync.dma_start(out=w2_sb[:, l * 128:(l + 1) * 128],
                                  in_=t_w2[l * HID:(l + 1) * HID, :])

            hT = sp.tile([128, SLICE], dt.float32)
            cbT = sp.tile([128, SLICE], dt.float32)

            def t512(tag="t512"):
                pool = wp if tag == "t512" else wp2
                return pool.tile([128, 512], dt.float32, tag=tag, name=tag)

            def learner(lidx, zin):
                z1 = pp.tile([HID, 512], dt.float32, tag="zy")
                nc.tensor.matmul(z1[:], w1_sb[:, lidx * HID:(lidx + 1) * HID],
                                 zin[:])
                zb = t512()
                nc.vector.tensor_scalar(zb[:HID, :], z1[:],
                                        b1_sb[:, lidx:lidx + 1], None, OP.add)
                zs = t512()
                nc.vector.tensor_scalar(zs[:HID, :], zb[:HID, :], 0.2, None,
                                        OP.mult)
                z = t512()
                nc.vector.tensor_tensor(z[:HID, :], zb[:HID, :], zs[:HID, :],
                                        OP.max)
                y = pp.tile([128, 512], dt.float32, tag="zy")
                nc.tensor.matmul(y[:], w2_sb[:, lidx * 128:(lidx + 1) * 128],
                                 z[:HID, :])
                ee = t512()
                nc.scalar.activation(ee[:], y[:], AF.Exp,
                                     bias=pb_sb[:, 18 + lidx:19 + lidx], scale=-1.0)
                sps = t512()
                nc.scalar.activation(sps[:], ee[:], AF.Ln,
                                     bias=colc_sb[:, 0:1])
                nw = t512()
                nc.scalar.activation(nw[:], sps[:], AF.Exp, scale=-1.0)
                return nw

            def writeback(g, src_ap):
                stg = wp2.tile([128, 4, 128], dt.bfloat16, tag="stage")
                for w4 in range(4):
                    tr = pp.tile([128, 128], dt.float32, tag="zy")
                    nc.tensor.transpose(tr[:], src_ap[:, w4 * 128:(w4 + 1) * 128],
                                        ident_sb[:])
                    nc.vector.tensor_copy(stg[:, w4, :], tr[:])
                dst = h_stage[g * 512:(g + 1) * 512, :]
                dst = dst.rearrange("(w p) f -> p w f", p=128)
                nc.sync.dma_start(out=dst, in_=stg[:])

            # ---- prologue: h0, codebank, initial allgather ----
            for g in range(NGRP):
                cols = slice(g * 512, (g + 1) * 512)
                x9 = t512()
                nc.sync.dma_start(out=x9[:9, :], in_=t_x[:, cols])
                up = pp.tile([8, 512], dt.float32, tag="st")
                nc.tensor.matmul(up[:], wx_sb[:], x9[:9, :])
                nf = t512()
                nc.vector.tensor_copy(nf[0:8, :], up[:])
                nc.sync.dma_start(out=nf[8:16, :], in_=t_tg[:, cols])
                nc.sync.dma_start(out=nf[16:17, :], in_=t_x[8:9, cols])
                h0p = pp2.tile([128, 512], dt.float32, tag="h1")
                nc.tensor.matmul(h0p[:], wenc_sb[:], nf[:17, :])
                h0 = t512()
                nc.vector.tensor_copy(h0[:], h0p[:])
                nw = learner(0, h0)
                nc.vector.tensor_tensor(hT[:, cols], h0[:], nw[:], OP.mult)
                nc.vector.tensor_tensor(cbT[:, cols], hT[:, cols], nw[:], OP.mult)
                writeback(g, hT[:, cols])

            nc.gpsimd.collective_compute(
                "AllGather", OP.bypass, replica_groups=rg,
                ins=[h_stage[:, :]], outs=[h_full[:, :]])

            # ---- prologue: edge embeddings to DRAM (edge-major bf16) ----
            for j0 in range(0, totch, 8):
                j1 = min(j0 + 8, totch)
                a9 = ep1.tile([9, 8 * 128], dt.float32, tag="a9")
                nc.sync.dma_start(out=a9[:, :(j1 - j0) * 128],
                                  in_=t_attr[:, j0 * 128:j1 * 128])
                for jb in range(j0, j1, 4):
                    je = min(jb + 4, j1)
                    ep_ps = pp2.tile([128, 512], dt.float32, tag="h1")
                    for k in range(jb, je):
                        off = (k - j0) * 128
                        nc.tensor.matmul(
                            ep_ps[:, (k - jb) * 128:(k - jb + 1) * 128],
                            a9[:, off:off + 128], wedge_sb[:])
                    es = wp2.tile([128, 4, 128], dt.bfloat16, tag="stage")
                    nc.vector.tensor_copy(
                        es[:, :je - jb, :],
                        ep_ps[:, :(je - jb) * 128].rearrange(
                            "p (j f) -> p j f", f=128))
                    nc.sync.dma_start(out=emb_dram[:, jb:je, :],
                                      in_=es[:, :je - jb, :])

            # ---- layers ----
            for l in range(L):
                tl = tvals[l]
                for g in range(NGRP):
                    cols = slice(g * 512, (g + 1) * 512)

                    ndN = pp2.tile([128, 512], dt.float32, tag="num")
                    ndD = pp2.tile([128, 512], dt.float32, tag="den")
                    for half in range(2):
                        h0, h1, runs = half_meta[2 * g + half]
                        ne = h1 - h0

                        hs = epool.tile([128, hmax], dt.bfloat16, tag="eA")
                        j0 = h0 // 128
                        ncol = (h1 - h0) // 128
                        for b in range(NBK):
                            bruns = [r for r in runs if r[0] == b]
                            S0 = bruns[0][3]
                            S1 = bruns[-1][3] + bruns[-1][2] * 128
                            n = S1 - S0
                            nc.gpsimd.dma_gather(
                                out_ap=hs[:, S0 - h0:S1 - h0].rearrange(
                                    "p (j f) -> p j f", f=128),
                                in_ap=h_full[b * BANK:(b + 1) * BANK, :],
                                idxs_ap=i16_sb[:, S0 // 16:S1 // 16],
                                num_idxs=n,
                                num_idxs_reg=n,
                                elem_size=128,
                            )
                        if DEBUG_DUMP and l == 0 and g == 0 and half == 0:
                            hd = wp2.tile([128, hmax], dt.float32,
                                          tag="dbged", name="dbghs")
                            nc.vector.tensor_copy(hd[:, :ne], hs[:, :ne])
                            nc.sync.dma_start(out=t_dbg[:, :hmax], in_=hd[:])
                            hf_sb = wp2.tile([128, 512], dt.bfloat16,
                                             tag="dbghf", name="dbghf")
                            nc.sync.dma_start(
                                out=hf_sb[:].rearrange("p (a f) -> p a f", f=128),
                                in_=h_full[0:512, :].rearrange(
                                    "(a p) f -> p a f", p=128))
                            hf2 = wp2.tile([128, 512], dt.float32,
                                           tag="dbghf2", name="dbghf2")
                            nc.vector.tensor_copy(hf2[:], hf_sb[:])
                            nc.sync.dma_start(
                                out=t_dbg[:, SLICE:SLICE + 512], in_=hf2[:])
                        eb = ep1.tile([128, hmax], dt.bfloat16, tag="eB")
                        nc.sync.dma_start(
                            out=eb[:, :ne].rearrange("p (j f) -> p j f", f=128),
                            in_=emb_dram[:, j0:j0 + ncol, :])
                        nc.vector.tensor_tensor(hs[:, :ne], hs[:, :ne],
                                                eb[:, :ne], OP.add)
                        nc.scalar.activation(eb[:, :ne], hs[:, :ne], AF.Relu)
                        ev = epool.tile([128, hmax], dt.bfloat16, tag="eC")
                        nc.scalar.activation(ev[:, :ne], eb[:, :ne], AF.Exp,
                                             scale=tl)
                        em = epool.tile([128, hmax], dt.bfloat16, tag="eD")
                        nc.gpsimd.tensor_tensor(em[:, :ne], ev[:, :ne],
                                                eb[:, :ne], OP.mult)

                        oh = epool.tile([128, hmax], dt.bfloat16, tag="oh")
                        iap = iota_sb[:]
                        i3 = bass.AP(iap.tensor, iap.offset,
                                     [iap.ap[0], [0, ncol], iap.ap[1]])
                        dap = dloc_sb[:, j0:j0 + ncol]
                        d3 = bass.AP(dap.tensor, dap.offset,
                                     [dap.ap[0], dap.ap[1], [0, 128]])
                        nc.vector.tensor_tensor(
                            oh[:, :ne].rearrange("p (j f) -> p j f", f=128),
                            i3, d3, OP.is_equal)
                        # chunk matmuls; window w accumulation spans its
                        # per-bank runs (start at first, stop at last)
                        for (b, w, kw, S0) in sorted(runs,
                                                     key=lambda r: (r[1], r[0])):
                            w4 = w - 4 * g
                            for k in range(kw):
                                off = S0 - h0 + k * 128
                                st = (b == 0 and k == 0)
                                sp = (b == NBK - 1 and k == kw - 1)
                                nc.tensor.matmul(
                                    ndN[:, w4 * 128:(w4 + 1) * 128],
                                    em[:, off:off + 128],
                                    oh[:, off:off + 128],
                                    start=st, stop=sp,
                                    skip_group_check=True)
                                nc.tensor.matmul(
                                    ndD[:, w4 * 128:(w4 + 1) * 128],
                                    ev[:, off:off + 128],
                                    oh[:, off:off + 128],
                                    start=st, stop=sp,
                                    skip_group_check=True)

                    lnd = t512()
                    nc.scalar.activation(lnd[:], ndD[:], AF.Ln,
                                         bias=colc_sb[:, 1:2])
                    rec = t512()
                    nc.scalar.activation(rec[:], lnd[:], AF.Exp, scale=-1.0)
                    hh = t512()
                    nc.vector.tensor_tensor(hh[:], ndN[:], rec[:], OP.mult)
                    nc.vector.tensor_tensor(hh[:], hh[:], hT[:, cols], OP.add)
                    if False:
                        nd_sb = t512("dbgnum")
                        nc.vector.tensor_copy(nd_sb[:], ndN[:])
                        nc.sync.dma_start(out=t_dbg[:, g * 512:(g + 1) * 512],
                                          in_=nd_sb[:])
                        dd_sb = t512("dbgnum")
                        nc.vector.tensor_copy(dd_sb[:], ndD[:])
                        nc.sync.dma_start(
                            out=t_dbg[:, SLICE + g * 512:SLICE + (g + 1) * 512],
                            in_=dd_sb[:])

                    h1p = pp2.tile([128, 512], dt.float32, tag="h1")
                    nc.tensor.matmul(h1p[:], gcnw_sb[:, l * 128:(l + 1) * 128],
                                     hh[:])
                    h1 = t512()
                    nc.scalar.activation(h1[:], h1p[:], AF.Identity,
                                         bias=pb_sb[:, l:l + 1])
                    sq = t512()
                    nc.scalar.activation(sq[:], h1[:], AF.Square)
                    sts = pp.tile([1, 512], dt.float32, tag="st")
                    stq = pp.tile([1, 512], dt.float32, tag="st2")
                    nc.tensor.matmul(sts[:], colc_sb[:, 0:1], h1[:])
                    nc.tensor.matmul(stq[:], colc_sb[:, 0:1], sq[:])
                    m2 = t512()
                    nc.scalar.activation(m2[:1, :], sts[:], AF.Square,
                                         scale=float(1.0 / np.sqrt(128.0)))
                    dv = t512()
                    nc.vector.tensor_tensor(dv[:1, :], stq[:], m2[:1, :],
                                            OP.subtract)
                    lnv = t512()
                    nc.scalar.activation(lnv[:1, :], dv[:1, :], AF.Ln,
                                         bias=colc_sb[:1, 2:3],
                                         scale=float(1.0 / 128.0))
                    rst = t512()
                    nc.scalar.activation(rst[:1, :], lnv[:1, :], AF.Exp,
                                         scale=-0.5)
                    tmu = t512()
                    nc.vector.tensor_tensor(tmu[:1, :], sts[:], rst[:1, :],
                                            OP.mult)
                    aB = pp.tile([128, 512], dt.float32, tag="ab")
                    cB = pp.tile([128, 512], dt.float32, tag="cb")
                    nc.tensor.matmul(aB[:], rowc_sb[:, 0:128], rst[:1, :])
                    nc.tensor.matmul(cB[:], rowc_sb[:, 128:256], tmu[:1, :])
                    hn = t512("hn")
                    nc.vector.tensor_tensor(hn[:], h1[:], aB[:], OP.mult)
                    nc.vector.tensor_tensor(hn[:], hn[:], cB[:], OP.add)
                    nc.vector.tensor_scalar(hn[:], hn[:], pb_sb[:, 6 + l:7 + l],
                                            pb_sb[:, 12 + l:13 + l],
                                            OP.mult, OP.add)
                    nc.vector.tensor_scalar(hn[:], hn[:], 0.0, None, OP.max)

                    zin = t512()
                    nc.vector.tensor_tensor(zin[:], hn[:], cbT[:, cols], OP.add)
                    nw = learner(l + 1, zin)
                    hf = t512("hf")
                    nc.vector.tensor_tensor(hf[:], hn[:], nw[:], OP.mult)
                    qq = t512("qq")
                    nc.vector.tensor_tensor(qq[:], cbT[:, cols], nw[:], OP.mult)
                    nc.vector.tensor_tensor(cbT[:, cols], cbT[:, cols], hf[:],
                                            OP.add)
                    nc.vector.tensor_tensor(hT[:, cols], cbT[:, cols], qq[:],
                                            OP.subtract)
                    writeback(g, hT[:, cols])

                nc.gpsimd.collective_compute(
                    "AllGather", OP.bypass, replica_groups=rg,
                    ins=[h_stage[:, :]], outs=[h_full[:, :]])

            # ---- epilogue ----
            for g in range(NGRP):
                cols = slice(g * 512, (g + 1) * 512)
                op_ps = pp2.tile([TASKS, 512], dt.float32, tag="h1")
                nc.tensor.matmul(op_ps[:], wpred_sb[:], cbT[:, cols])
                ot = t512("ot")
                nc.vector.tensor_scalar(ot[:TASKS, :], op_ps[:],
                                        pb_sb[:TASKS, 25:26], None, OP.add)
                for w4 in range(4):
                    tr = pp.tile([128, TASKS], dt.float32, tag="zy")
                    nc.tensor.transpose(tr[:], ot[:TASKS,
                                                  w4 * 128:(w4 + 1) * 128],
                                        ident_sb[:TASKS, :TASKS])
                    os_ = t512("ot")
                    nc.vector.tensor_copy(os_[:, :TASKS], tr[:])
                    r0 = g * 512 + w4 * 128
                    nc.sync.dma_start(out=t_out[r0:r0 + 128, :],
                                      in_=os_[:, :TASKS])

    return nc


# ---------------- entry point ----------------
def kernel(**inputs):
    from concourse.bass_utils import run_bass_kernel_spmd

    pk = _pack_graph(np.asarray(inputs["edge_index"]))
    maps = _build_inputs(inputs, pk)
    tvals = [float(v) for v in np.asarray(inputs["gcn_t"], np.float32)]

    nc = _build_program(pk, tvals)
    if not nc.is_finalized():
        nc.finalize()
    trace = bool(int(os.environ.get("KERNEL_PROFILE", "0")))
    res = run_bass_kernel_spmd(nc, maps, list(range(C)), trace=trace)
    kernel.exec_time_ns = res.exec_time_ns
    kernel.profile_json = res.profile_json

    out = np.zeros((N, TASKS), np.float32)
    for c in range(C):
        oc = np.asarray(res.results[c]["out"], np.float32)
        pm = pk["perm"][c]
        valid = pm >= 0
        out[pm[valid]] = oc[valid]
    if DEBUG_DUMP:
        kernel.dbg = [np.asarray(res.results[c].get("dbg")) for c in range(C)]
        kernel.pk = pk
    return out



# revision 6
# speedup vs baseline: 2.0084x; 2.0084x over previous
"""Trainium2 Bass kernel for nn_DeepNoSAF (6-layer GENConv-style GNN).

Sharding: nodes partitioned across 8 cores by dst range; each core owns the
incoming edges of its nodes (host sorts/pads edges into per-window chunks of
128).  Node state h is replicated in HBM (bf16) for the per-edge gather
(indirect DMA); updated slices are exchanged per layer with an AllGather.
Per-channel segment softmax is computed with one-hot matmuls accumulating
num=sum(e*m), den=sum(e) in PSUM per 128-dst window (max-subtraction skipped;
the +1e-16 denominator keeps empty segments at 0).  Node phase runs F-major
(weights stationary); LayerNorm stats via ones-matmuls, broadcasts via K=1
matmuls; all transcendentals use one ACT table set {exp, ln, lrelu, square}:
sigmoid(x)=exp(-ln(1+exp(-x))), rsqrt(x)=exp(-0.5*ln(x)).
"""

import os
import sys

sys.path.insert(0, "/opt/trn_rl_repo")

import numpy as np
import ml_dtypes

# ---------------- problem constants (hardcoded per spec) ----------------
N = 100000
E = 625000
F = 128
L = 6
HID = 64
NTOT = 200000
TASKS = 112
LN_EPS = 1e-5
C = 8                      # cores
S_NODES = N // C           # 12500 owned nodes per core
NWIN = 100                 # windows per core
WIN = 128                  # dst slots per window
SLICE = NWIN * WIN         # 12800 node slots per core
NGRP = NWIN // 4           # 25 groups of 4 windows (512 node cols)

BF16 = ml_dtypes.bfloat16


# ---------------- host-side graph packing ----------------
NBK = 4


def _pack_graph(edge_index):
    src = np.asarray(edge_index[0], dtype=np.int64)
    dst = np.asarray(edge_index[1], dtype=np.int64)
    core_of = dst // S_NODES
    BANK = C * SLICE // NBK

    deg = np.bincount(dst, minlength=N)
    node_win = np.full(N, -1, np.int32)
    node_pos = np.full(N, -1, np.int32)
    loads = np.zeros((C, NWIN), np.int64)

    avg = int(deg.sum()) // (C * NWIN)
    base_cap = max(WIN, (avg // WIN) * WIN)
    n_hi = max(1, (NWIN * 3) // 10)
    targets = np.array([base_cap + WIN] * n_hi + [base_cap] * (NWIN - n_hi),
                       np.int64)
    NEG = np.iinfo(np.int64).min
    for c in range(C):
        lo = c * S_NODES
        nodes = lo + np.argsort(-deg[lo:lo + S_NODES], kind="stable")
        counts = np.zeros(NWIN, np.int32)
        ld = loads[c]
        for n in nodes:
            room = targets - ld
            room[counts >= WIN] = NEG
            w = int(np.argmax(room))
            node_win[n] = w
            node_pos[n] = counts[w]
            counts[w] += 1
            ld[w] += deg[n]

    perm = np.full((C, SLICE), -1, np.int64)
    alln = np.arange(N)
    slot_global = node_win[alln] * WIN + node_pos[alln]
    perm[(alln // S_NODES), slot_global] = alln
    hrow = (alln // S_NODES) * SLICE + slot_global

    ew = node_win[dst]
    ebk = (hrow[src] // BANK).astype(np.int64)
    cnt = np.zeros((C, NWIN, NBK), np.int64)
    np.add.at(cnt, (core_of, ew, ebk), 1)
    K = np.maximum(1, -(-cnt.max(axis=0) // WIN))

    wb_base = np.zeros((NWIN, NBK), np.int64)
    off = 0
    half_meta = []
    for wlo in range(0, NWIN, 2):
        h0 = off
        runs = []
        for b in range(NBK):
            for w in (wlo, wlo + 1):
                wb_base[w, b] = off
                runs.append((b, w, int(K[w, b]), off))
                off += int(K[w, b]) * WIN
        half_meta.append((h0, off, runs))
    nslot = int(off)
    totch = nslot // WIN

    order = np.lexsort((np.arange(E), ebk, ew, core_of))
    src_s, dst_s, core_s = src[order], dst[order], core_of[order]
    grp_key = core_s * (NWIN * NBK) + ew[order] * NBK + ebk[order]
    uniq, start_idx, cnts = np.unique(grp_key, return_index=True,
                                      return_counts=True)
    run = np.arange(E) - np.repeat(start_idx, cnts)
    slot_of_edge = wb_base[ew[order], ebk[order]] + run

    src_img = np.zeros((C, totch * WIN), np.int32)
    dloc_img = np.full((C, totch * WIN), -1.0, np.float32)
    eperm = np.full((C, nslot), -1, np.int64)
    i16 = np.zeros((C, nslot), np.int16)
    src_img[core_s, slot_of_edge] = hrow[src_s].astype(np.int32)
    i16.reshape(C, -1)[core_s, slot_of_edge] = (hrow[src_s] % BANK).astype(np.int16)
    dloc_img[core_s, slot_of_edge] = node_pos[dst_s].astype(np.float32)
    eperm[core_s, slot_of_edge] = order

    idx32_img = np.ascontiguousarray(
        src_img.reshape(C, totch, WIN).transpose(0, 2, 1))
    dl_img = np.ascontiguousarray(
        dloc_img.reshape(C, totch, WIN).transpose(0, 2, 1))
    idx16_img = np.zeros((C, 128, nslot // 16), np.int16)
    sl = np.arange(nslot)
    for k in range(8):
        idx16_img[:, sl % 16 + 16 * k, sl // 16] = i16

    return dict(K=K, half_meta=half_meta, nslot=nslot, totch=totch,
                perm=perm, idx32_img=idx32_img, idx16_img=idx16_img,
                dl_img=dl_img, eperm=eperm, BANK=BANK)


def _build_inputs(inputs, pk):
    x = np.asarray(inputs["x"], np.float32)
    node_index = np.asarray(inputs["node_index"]).astype(np.int64)
    edge_attr = np.asarray(inputs["edge_attr"], np.float32)
    table = np.asarray(inputs["node_features_table"], np.float32)

    perm, eperm = pk["perm"], pk["eperm"]
    nslot = pk["nslot"]

    w_enc = np.asarray(inputs["W_enc"], np.float32)
    b_enc = np.asarray(inputs["b_enc"], np.float32)
    w_ohe = np.asarray(inputs["W_ohe"], np.float32)
    b_ohe = np.asarray(inputs["b_ohe"], np.float32)
    w_edge = np.asarray(inputs["W_edge"], np.float32)
    b_edge = np.asarray(inputs["b_edge"], np.float32)

    # nf row order on device: [u (x@W_ohe+b_ohe) rows 0-7; tg rows 8-15; ones]
    wenc_aug = np.concatenate([w_enc[8:16], w_enc[0:8], b_enc[None, :]], 0)
    wx_aug = np.concatenate([w_ohe, b_ohe[None, :]], 0)            # [9,8]
    wedge_aug = np.concatenate([w_edge, b_edge[None, :]], 0)       # [9,128]

    gcnw = np.asarray(inputs["gcn_W"], np.float32).reshape(-1, F)
    w1 = np.asarray(inputs["learner_W1"], np.float32).reshape(-1, HID)
    w2 = np.asarray(inputs["learner_W2"], np.float32).reshape(-1, F)
    wpred = np.asarray(inputs["W_pred"], np.float32)

    pb = np.zeros((F, 26), np.float32)
    pb[:, 0:6] = np.asarray(inputs["gcn_b"], np.float32).T
    pb[:, 6:12] = np.asarray(inputs["ln_gamma"], np.float32).T
    pb[:, 12:18] = np.asarray(inputs["ln_beta"], np.float32).T
    pb[:, 18:25] = -np.asarray(inputs["learner_b2"], np.float32).T
    pb[:TASKS, 25] = np.asarray(inputs["b_pred"], np.float32)
    b1s = np.ascontiguousarray(np.asarray(inputs["learner_b1"], np.float32).T)

    iota = np.broadcast_to(np.arange(128, dtype=np.float32)[None, :],
                           (128, 128)).astype(BF16)
    ident = np.eye(128, dtype=np.float32)
    rowc = np.zeros((1, 256), np.float32)
    rowc[0, :128] = 1.0
    rowc[0, 128:] = -1.0 / 128.0
    colc = np.zeros((128, 4), np.float32)
    colc[:, 0] = 1.0
    colc[:, 1] = 1e-16
    colc[:, 2] = LN_EPS

    maps = []
    for c in range(C):
        pm = perm[c]
        valid = pm >= 0
        xs = np.zeros((SLICE, 8), np.float32)
        xs[valid] = x[pm[valid]]
        tg = np.zeros((SLICE, 8), np.float32)
        tg[valid] = table[node_index[pm[valid]]]
        xT9 = np.zeros((9, SLICE), np.float32)
        xT9[:8] = xs.T
        xT9[8] = 1.0
        tgT = np.ascontiguousarray(tg.T)

        ep = eperm[c]
        ev = ep >= 0
        ea = np.zeros((nslot, 8), np.float32)
        ea[ev] = edge_attr[ep[ev]]
        attrT = np.zeros((9, nslot), np.float32)
        attrT[:8] = ea.T
        attrT[8, :] = 1.0

        maps.append({
            "idx16": pk["idx16_img"][c], "dloc": pk["dl_img"][c],
            "attrT": attrT, "xT9": xT9, "tgT": tgT,
            "iota": np.ascontiguousarray(iota), "ident": ident,
            "rowc": rowc, "colc": colc,
            "wenc": wenc_aug, "wx": wx_aug, "wedge": wedge_aug,
            "gcnw": gcnw, "w1": w1, "w2": w2, "wpred": wpred,
            "pb": pb, "b1s": b1s,
        })
    return maps


DEBUG_DUMP = False


# ---------------- bass program ----------------
def _build_program(pk, tvals):
    import concourse.bass as bass
    import concourse.tile as tile
    from concourse import bacc, mybir
    import concourse.hw_specs as hw_specs

    dt = mybir.dt
    AF = mybir.ActivationFunctionType
    OP = mybir.AluOpType

    class OneTableBacc(bacc.Bacc):
        """All activation funcs used here (Exp, Ln, Relu, Identity, Square)
        live in the natural_log_exp_and_others table set, but the stock
        chooser picks the first set containing each func (exp_and_others vs
        natural_log), thrashing ~2.7us ACT_TABLE_LOAD+DRAIN per switch.
        Strip Exp/Ln from every other set so one load serves the kernel."""

        def insert_act_table_loads(self):
            has_activation = any(
                isinstance(i, mybir.InstActivation)
                for b in self.main_func.blocks
                for i in b.instructions
            )
            if not has_activation:
                return
            tables = list(hw_specs.get_activation_tables(self.m.arch).items())
            assert any(n == "natural_log_exp_and_others" for n, _ in tables)
            filtered = [
                (n, f) if n == "natural_log_exp_and_others"
                else (n, f - {AF.Exp, AF.Ln})
                for n, f in tables
            ]
            bacc._bass_rust.insert_act_table_loads(self, filtered)

    K, half_meta = pk["K"], pk["half_meta"]
    nslot, totch, BANK = pk["nslot"], pk["totch"], pk["BANK"]
    HROWS = C * SLICE

    nc = OneTableBacc(num_devices=C)

    t_idx16 = nc.dram_tensor("idx16", [128, nslot // 16], dt.int16,
                             kind="ExternalInput")
    t_dloc = nc.dram_tensor("dloc", [128, totch], dt.float32, kind="ExternalInput")
    t_attr = nc.dram_tensor("attrT", [9, nslot], dt.float32, kind="ExternalInput")
    t_x = nc.dram_tensor("xT9", [9, SLICE], dt.float32, kind="ExternalInput")
    t_tg = nc.dram_tensor("tgT", [8, SLICE], dt.float32, kind="ExternalInput")
    t_iota = nc.dram_tensor("iota", [128, 128], dt.bfloat16, kind="ExternalInput")
    t_ident = nc.dram_tensor("ident", [128, 128], dt.float32, kind="ExternalInput")
    t_rowc = nc.dram_tensor("rowc", [1, 256], dt.float32, kind="ExternalInput")
    t_colc = nc.dram_tensor("colc", [128, 4], dt.float32, kind="ExternalInput")
    t_wenc = nc.dram_tensor("wenc", [17, 128], dt.float32, kind="ExternalInput")
    t_wx = nc.dram_tensor("wx", [9, 8], dt.float32, kind="ExternalInput")
    t_wedge = nc.dram_tensor("wedge", [9, 128], dt.float32, kind="ExternalInput")
    t_gcnw = nc.dram_tensor("gcnw", [6 * F, F], dt.float32, kind="ExternalInput")
    t_w1 = nc.dram_tensor("w1", [7 * F, HID], dt.float32, kind="ExternalInput")
    t_w2 = nc.dram_tensor("w2", [7 * HID, F], dt.float32, kind="ExternalInput")
    t_wpred = nc.dram_tensor("wpred", [F, TASKS], dt.float32, kind="ExternalInput")
    t_pb = nc.dram_tensor("pb", [128, 26], dt.float32, kind="ExternalInput")
    t_b1 = nc.dram_tensor("b1s", [HID, 7], dt.float32, kind="ExternalInput")
    t_out = nc.dram_tensor("out", [SLICE, TASKS], dt.float32, kind="ExternalOutput")
    t_dbg = None
    if DEBUG_DUMP:
        t_dbg = nc.dram_tensor("dbg", [128, 2 * SLICE], dt.float32,
                               kind="ExternalOutput")


    h_full = nc.dram_tensor("h_full", [HROWS, F], dt.bfloat16,
                            addr_space="Shared")
    h_stage = nc.dram_tensor("h_stage", [SLICE, F], dt.bfloat16)
    emb_dram = nc.dram_tensor("emb_dram", [128, totch, 128], dt.bfloat16)

    rg = [list(range(C))]
    # edge phase runs per half-group (2 windows) to bound SBUF tiles
    hmax = max(h1 - h0 for (h0, h1, _) in half_meta)

    with tile.TileContext(nc) as tc:
        with (
            tc.tile_pool(name="const", bufs=1) as cp,
            tc.tile_pool(name="state", bufs=1) as sp,
            tc.tile_pool(name="work", bufs=8) as wp,
            tc.tile_pool(name="work2", bufs=2) as wp2,
            tc.tile_pool(name="edge", bufs=2) as epool,
            tc.tile_pool(name="edge1", bufs=1) as ep1,
            tc.tile_pool(name="psum", bufs=1, space="PSUM") as pp,
            tc.tile_pool(name="psum2", bufs=1, space="PSUM") as pp2,
        ):
            def load_const(tt, shape, dtype):
                s = cp.tile(shape, dtype, tag=tt.name, name=tt.name + "_sb")
                nc.sync.dma_start(out=s[:], in_=tt[:])
                return s

            iota_sb = load_const(t_iota, [128, 128], dt.bfloat16)
            ident_sb = load_const(t_ident, [128, 128], dt.float32)
            rowc_sb = load_const(t_rowc, [1, 256], dt.float32)
            colc_sb = load_const(t_colc, [128, 4], dt.float32)
            wenc_sb = load_const(t_wenc, [17, 128], dt.float32)
            wx_sb = load_const(t_wx, [9, 8], dt.float32)
            wedge_sb = load_const(t_wedge, [9, 128], dt.float32)
            wpred_sb = load_const(t_wpred, [128, TASKS], dt.float32)
            pb_sb = load_const(t_pb, [128, 26], dt.float32)
            b1_sb = load_const(t_b1, [HID, 7], dt.float32)
            i16_sb = load_const(t_idx16, [128, nslot // 16], dt.int16)
            dloc_sb = load_const(t_dloc, [128, totch], dt.float32)

            gcnw_sb = cp.tile([128, 6 * 128], dt.float32)
            w1_sb = cp.tile([128, 7 * HID], dt.float32)
            w2_sb = cp.tile([HID, 7 * 128], dt.float32)
            for l in range(6):
                nc.sync.dma_start(out=gcnw_sb[:, l * 128:(l + 1) * 128],
                                  in_=t_gcnw[l * 128:(l + 1) * 128, :])
            for l in range(7):
                nc.sync.dma_start(out=w1_sb[:, l * HID:(l + 1) * HID],
                                  in_=t_w1[l * F:(l + 1) * F, :])
                nc.sync.dma_start(out=w2_sb[:, l * 128:(l + 1) * 128],
                                  in_=t_w2[l * HID:(l + 1) * HID, :])

            hT = sp.tile([128, SLICE], dt.float32)
            cbT = sp.tile([128, SLICE], dt.float32)

            def t512(tag="t512"):
                pool = wp if tag == "t512" else wp2
                return pool.tile([128, 512], dt.float32, tag=tag, name=tag)

            def learner(lidx, zin):
                z1 = pp.tile([HID, 512], dt.float32, tag="zy")
                nc.tensor.matmul(z1[:], w1_sb[:, lidx * HID:(lidx + 1) * HID],
                                 zin[:])
                zb = t512()
                nc.vector.tensor_scalar(zb[:HID, :], z1[:],
                                        b1_sb[:, lidx:lidx + 1], None, OP.add)
                zs = t512()
                nc.vector.tensor_scalar(zs[:HID, :], zb[:HID, :], 0.2, None,
                                        OP.mult)
                z = t512()
                nc.vector.tensor_tensor(z[:HID, :], zb[:HID, :], zs[:HID, :],
                                        OP.max)
                y = pp.tile([128, 512], dt.float32, tag="zy")
                nc.tensor.matmul(y[:], w2_sb[:, lidx * 128:(lidx + 1) * 128],
                                 z[:HID, :])
                ee = t512()
                nc.scalar.activation(ee[:], y[:], AF.Exp,
                                     bias=pb_sb[:, 18 + lidx:19 + lidx], scale=-1.0)
                sps = t512()
                nc.scalar.activation(sps[:], ee[:], AF.Ln,
                                     bias=colc_sb[:, 0:1])
                nw = t512()
                nc.scalar.activation(nw[:], sps[:], AF.Exp, scale=-1.0)
                return nw

            def writeback(g, src_ap):
                stg = wp2.tile([128, 4, 128], dt.bfloat16, tag="stage")
                for w4 in range(4):
                    tr = pp.tile([128, 128], dt.float32, tag="zy")
                    nc.tensor.transpose(tr[:], src_ap[:, w4 * 128:(w4 + 1) * 128],
                                        ident_sb[:])
                    nc.vector.tensor_copy(stg[:, w4, :], tr[:])
                dst = h_stage[g * 512:(g + 1) * 512, :]
                dst = dst.rearrange("(w p) f -> p w f", p=128)
                nc.sync.dma_start(out=dst, in_=stg[:])

            # ---- prologue: h0, codebank, initial allgather ----
            for g in range(NGRP):
                cols = slice(g * 512, (g + 1) * 512)
                x9 = t512()
                nc.sync.dma_start(out=x9[:9, :], in_=t_x[:, cols])
                up = pp.tile([8, 512], dt.float32, tag="st")
                nc.tensor.matmul(up[:], wx_sb[:], x9[:9, :])
                nf = t512()
                nc.vector.tensor_copy(nf[0:8, :], up[:])
                nc.sync.dma_start(out=nf[8:16, :], in_=t_tg[:, cols])
                nc.sync.dma_start(out=nf[16:17, :], in_=t_x[8:9, cols])
                h0p = pp2.tile([128, 512], dt.float32, tag="h1")
                nc.tensor.matmul(h0p[:], wenc_sb[:], nf[:17, :])
                h0 = t512()
                nc.vector.tensor_copy(h0[:], h0p[:])
                nw = learner(0, h0)
                nc.vector.tensor_tensor(hT[:, cols], h0[:], nw[:], OP.mult)
                nc.vector.tensor_tensor(cbT[:, cols], hT[:, cols], nw[:], OP.mult)
                writeback(g, hT[:, cols])

            nc.gpsimd.collective_compute(
                "AllGather", OP.bypass, replica_groups=rg,
                ins=[h_stage[:, :]], outs=[h_full[:, :]])

            # ---- prologue: edge embeddings to DRAM (edge-major bf16) ----
            for j0 in range(0, totch, 8):
                j1 = min(j0 + 8, totch)
                a9 = ep1.tile([9, 8 * 128], dt.float32, tag="a9")
                nc.sync.dma_start(out=a9[:, :(j1 - j0) * 128],
                                  in_=t_attr[:, j0 * 128:j1 * 128])
                for jb in range(j0, j1, 4):
                    je = min(jb + 4, j1)
                    ep_ps = pp2.tile([128, 512], dt.float32, tag="h1")
                    for k in range(jb, je):
                        off = (k - j0) * 128
                        nc.tensor.matmul(
                            ep_ps[:, (k - jb) * 128:(k - jb + 1) * 128],
                            a9[:, off:off + 128], wedge_sb[:])
                    es = wp2.tile([128, 4, 128], dt.bfloat16, tag="stage")
                    nc.vector.tensor_copy(
                        es[:, :je - jb, :],
                        ep_ps[:, :(je - jb) * 128].rearrange(
                            "p (j f) -> p j f", f=128))
                    nc.sync.dma_start(out=emb_dram[:, jb:je, :],
                                      in_=es[:, :je - jb, :])

            # ---- layers ----
            for l in range(L):
                tl = tvals[l]
                for g in range(NGRP):
                    cols = slice(g * 512, (g + 1) * 512)

                    ndN = pp2.tile([128, 512], dt.float32, tag="num")
                    ndD = pp2.tile([128, 512], dt.float32, tag="den")
                    for half in range(2):
                        h0, h1, runs = half_meta[2 * g + half]
                        ne = h1 - h0

                        hs = epool.tile([128, hmax], dt.bfloat16, tag="eA")
                        j0 = h0 // 128
                        ncol = (h1 - h0) // 128
                        for b in range(NBK):
                            bruns = [r for r in runs if r[0] == b]
                            S0 = bruns[0][3]
                            S1 = bruns[-1][3] + bruns[-1][2] * 128
                            n = S1 - S0
                            nc.gpsimd.dma_gather(
                                out_ap=hs[:, S0 - h0:S1 - h0].rearrange(
                                    "p (j f) -> p j f", f=128),
                                in_ap=h_full[b * BANK:(b + 1) * BANK, :],
                                idxs_ap=i16_sb[:, S0 // 16:S1 // 16],
                                num_idxs=n,
                                num_idxs_reg=n,
                                elem_size=128,
                            )
                        if DEBUG_DUMP and l == 0 and g == 0 and half == 0:
                            hd = wp2.tile([128, hmax], dt.float32,
                                          tag="dbged", name="dbghs")
                            nc.vector.tensor_copy(hd[:, :ne], hs[:, :ne])
                            nc.sync.dma_start(out=t_dbg[:, :hmax], in_=hd[:])
                            hf_sb = wp2.tile([128, 512], dt.bfloat16,
                                             tag="dbghf", name="dbghf")
                            nc.sync.dma_start(
                                out=hf_sb[:].rearrange("p (a f) -> p a f", f=128),
                                in_=h_full[0:512, :].rearrange(
                                    "(a p) f -> p a f", p=128))
                            hf2 = wp2.tile([128, 512], dt.float32,
                                           tag="dbghf2", name="dbghf2")
                            nc.vector.tensor_copy(hf2[:], hf_sb[:])
                            nc.sync.dma_start(
                                out=t_dbg[:, SLICE:SLICE + 512], in_=hf2[:])
                        eb = epool.tile([128, hmax], dt.bfloat16, tag="eB")
                        nc.sync.dma_start(
                            out=eb[:, :ne].rearrange("p (j f) -> p j f", f=128),
                            in_=emb_dram[:, j0:j0 + ncol, :])
                        nc.vector.tensor_tensor(hs[:, :ne], hs[:, :ne],
                                                eb[:, :ne], OP.add)
                        nc.scalar.activation(eb[:, :ne], hs[:, :ne], AF.Relu)
                        ev = epool.tile([128, hmax], dt.bfloat16, tag="eC")
                        nc.scalar.activation(ev[:, :ne], eb[:, :ne], AF.Exp,
                                             scale=tl)
                        em = epool.tile([128, hmax], dt.bfloat16, tag="eD")
                        nc.vector.tensor_tensor(em[:, :ne], ev[:, :ne],
                                                eb[:, :ne], OP.mult)

                        oh = epool.tile([128, hmax], dt.bfloat16, tag="oh")
                        iap = iota_sb[:]
                        i3 = bass.AP(iap.tensor, iap.offset,
                                     [iap.ap[0], [0, ncol], iap.ap[1]])
                        dap = dloc_sb[:, j0:j0 + ncol]
                        d3 = bass.AP(dap.tensor, dap.offset,
                                     [dap.ap[0], dap.ap[1], [0, 128]])
                        nc.vector.tensor_tensor(
                            oh[:, :ne].rearrange("p (j f) -> p j f", f=128),
                            i3, d3, OP.is_equal)
                        # chunk matmuls; window w accumulation spans its
                        # per-bank runs (start at first, stop at last)
                        for (b, w, kw, S0) in sorted(runs,
                                                     key=lambda r: (r[1], r[0])):
                            w4 = w - 4 * g
                            for k in range(kw):
                                off = S0 - h0 + k * 128
                                st = (b == 0 and k == 0)
                                sp = (b == NBK - 1 and k == kw - 1)
                                nc.tensor.matmul(
                                    ndN[:, w4 * 128:(w4 + 1) * 128],
                                    em[:, off:off + 128],
                                    oh[:, off:off + 128],
                                    start=st, stop=sp,
                                    skip_group_check=True)
                                nc.tensor.matmul(
                                    ndD[:, w4 * 128:(w4 + 1) * 128],
                                    ev[:, off:off + 128],
                                    oh[:, off:off + 128],
                                    start=st, stop=sp,
                                    skip_group_check=True)

                    lnd = t512()
                    nc.scalar.activation(lnd[:], ndD[:], AF.Ln,
                                         bias=colc_sb[:, 1:2])
                    rec = t512()
                    nc.scalar.activation(rec[:], lnd[:], AF.Exp, scale=-1.0)
                    hh = t512()
                    nc.vector.tensor_tensor(hh[:], ndN[:], rec[:], OP.mult)
                    nc.vector.tensor_tensor(hh[:], hh[:], hT[:, cols], OP.add)
                    if False:
                        nd_sb = t512("dbgnum")
                        nc.vector.tensor_copy(nd_sb[:], ndN[:])
                        nc.sync.dma_start(out=t_dbg[:, g * 512:(g + 1) * 512],
                                          in_=nd_sb[:])
                        dd_sb = t512("dbgnum")
                        nc.vector.tensor_copy(dd_sb[:], ndD[:])
                        nc.sync.dma_start(
                            out=t_dbg[:, SLICE + g * 512:SLICE + (g + 1) * 512],
                            in_=dd_sb[:])

                    h1p = pp2.tile([128, 512], dt.float32, tag="h1")
                    nc.tensor.matmul(h1p[:], gcnw_sb[:, l * 128:(l + 1) * 128],
                                     hh[:])
                    h1 = t512()
                    nc.scalar.activation(h1[:], h1p[:], AF.Identity,
                                         bias=pb_sb[:, l:l + 1])
                    sq = t512()
                    nc.scalar.activation(sq[:], h1[:], AF.Square)
                    sts = pp.tile([1, 512], dt.float32, tag="st")
                    stq = pp.tile([1, 512], dt.float32, tag="st2")
                    nc.tensor.matmul(sts[:], colc_sb[:, 0:1], h1[:])
                    nc.tensor.matmul(stq[:], colc_sb[:, 0:1], sq[:])
                    m2 = t512()
                    nc.scalar.activation(m2[:1, :], sts[:], AF.Square,
                                         scale=float(1.0 / np.sqrt(128.0)))
                    dv = t512()
                    nc.vector.tensor_tensor(dv[:1, :], stq[:], m2[:1, :],
                                            OP.subtract)
                    lnv = t512()
                    nc.scalar.activation(lnv[:1, :], dv[:1, :], AF.Ln,
                                         bias=colc_sb[:1, 2:3],
                                         scale=float(1.0 / 128.0))
                    rst = t512()
                    nc.scalar.activation(rst[:1, :], lnv[:1, :], AF.Exp,
                                         scale=-0.5)
                    tmu = t512()
                    nc.vector.tensor_tensor(tmu[:1, :], sts[:], rst[:1, :],
                                            OP.mult)
                    aB = pp.tile([128, 512], dt.float32, tag="ab")
                    cB = pp.tile([128, 512], dt.float32, tag="cb")
                    nc.tensor.matmul(aB[:], rowc_sb[:, 0:128], rst[:1, :])
                    nc.tensor.matmul(cB[:], rowc_sb[:, 128:256], tmu[:1, :])
                    hn = t512("hn")
                    nc.vector.tensor_tensor(hn[:], h1[:], aB[:], OP.mult)
                    nc.vector.tensor_tensor(hn[:], hn[:], cB[:], OP.add)
                    nc.vector.tensor_scalar(hn[:], hn[:], pb_sb[:, 6 + l:7 + l],
                                            pb_sb[:, 12 + l:13 + l],
                                            OP.mult, OP.add)
                    nc.vector.tensor_scalar(hn[:], hn[:], 0.0, None, OP.max)

                    zin = t512()
                    nc.vector.tensor_tensor(zin[:], hn[:], cbT[:, cols], OP.add)
                    nw = learner(l + 1, zin)
                    hf = t512("hf")
                    nc.vector.tensor_tensor(hf[:], hn[:], nw[:], OP.mult)
                    qq = t512("qq")
                    nc.vector.tensor_tensor(qq[:], cbT[:, cols], nw[:], OP.mult)
                    nc.vector.tensor_tensor(cbT[:, cols], cbT[:, cols], hf[:],
                                            OP.add)
                    nc.vector.tensor_tensor(hT[:, cols], cbT[:, cols], qq[:],
                                            OP.subtract)
                    writeback(g, hT[:, cols])

                nc.gpsimd.collective_compute(
                    "AllGather", OP.bypass, replica_groups=rg,
                    ins=[h_stage[:, :]], outs=[h_full[:, :]])

            # ---- epilogue ----
            for g in range(NGRP):
                cols = slice(g * 512, (g + 1) * 512)
                op_ps = pp2.tile([TASKS, 512], dt.float32, tag="h1")
                nc.tensor.matmul(op_ps[:], wpred_sb[:], cbT[:, cols])
                ot = t512("ot")
                nc.vector.tensor_scalar(ot[:TASKS, :], op_ps[:],
                                        pb_sb[:TASKS, 25:26], None, OP.add)
                for w4 in range(4):
                    tr = pp.tile([128, TASKS], dt.float32, tag="zy")
                    nc.tensor.transpose(tr[:], ot[:TASKS,
                                                  w4 * 128:(w4 + 1) * 128],
                                        ident_sb[:TASKS, :TASKS])
                    os_ = t512("ot")
                    nc.vector.tensor_copy(os_[:, :TASKS], tr[:])
                    r0 = g * 512 + w4 * 128
                    nc.sync.dma_start(out=t_out[r0:r0 + 128, :],
                                      in_=os_[:, :TASKS])

    return nc


# ---------------- entry point ----------------
def kernel(**inputs):
    from concourse.bass_utils import run_bass_kernel_spmd

    pk = _pack_graph(np.asarray(inputs["edge_index"]))
    maps = _build_inputs(inputs, pk)
    tvals = [float(v) for v in np.asarray(inputs["gcn_t"], np.float32)]

    nc = _build_program(pk, tvals)
    if not nc.is_finalized():
        nc.finalize()
    trace = bool(int(os.environ.get("KERNEL_PROFILE", "0")))
    res = run_bass_kernel_spmd(nc, maps, list(range(C)), trace=trace)
    kernel.exec_time_ns = res.exec_time_ns
    kernel.profile_json = res.profile_json

    out = np.zeros((N, TASKS), np.float32)
    for c in range(C):
        oc = np.asarray(res.results[c]["out"], np.float32)
        pm = pk["perm"][c]
        valid = pm >= 0
        out[pm[valid]] = oc[valid]
    if DEBUG_DUMP:
        kernel.dbg = [np.asarray(res.results[c].get("dbg")) for c in range(C)]
        kernel.pk = pk
    return out



# revision 8
# speedup vs baseline: 2.4148x; 1.2024x over previous
"""Trainium2 Bass kernel for nn_DeepNoSAF (6-layer GENConv-style GNN).

Sharding: nodes partitioned across 8 cores by dst range; each core owns the
incoming edges of its nodes (host sorts/pads edges into per-window chunks of
128).  Node state h is replicated in HBM (bf16) for the per-edge gather
(indirect DMA); updated slices are exchanged per layer with an AllGather.
Per-channel segment softmax is computed with one-hot matmuls accumulating
num=sum(e*m), den=sum(e) in PSUM per 128-dst window (max-subtraction skipped;
the +1e-16 denominator keeps empty segments at 0).  Node phase runs F-major
(weights stationary); LayerNorm stats via ones-matmuls, broadcasts via K=1
matmuls; all transcendentals use one ACT table set {exp, ln, lrelu, square}:
sigmoid(x)=exp(-ln(1+exp(-x))), rsqrt(x)=exp(-0.5*ln(x)).
"""

import os
import sys

sys.path.insert(0, "/opt/trn_rl_repo")

import numpy as np
import ml_dtypes

# ---------------- problem constants (hardcoded per spec) ----------------
N = 100000
E = 625000
F = 128
L = 6
HID = 64
NTOT = 200000
TASKS = 112
LN_EPS = 1e-5
C = 8                      # cores
S_NODES = N // C           # 12500 owned nodes per core
NWIN = 100                 # windows per core
WIN = 128                  # dst slots per window
SLICE = NWIN * WIN         # 12800 node slots per core
NGRP = NWIN // 4           # 25 groups of 4 windows (512 node cols)

BF16 = ml_dtypes.bfloat16


# ---------------- host-side graph packing ----------------
NBK = 4


def _pack_graph(edge_index):
    src = np.asarray(edge_index[0], dtype=np.int64)
    dst = np.asarray(edge_index[1], dtype=np.int64)
    core_of = dst // S_NODES
    BANK = C * SLICE // NBK

    deg = np.bincount(dst, minlength=N)
    node_win = np.full(N, -1, np.int32)
    node_pos = np.full(N, -1, np.int32)
    loads = np.zeros((C, NWIN), np.int64)

    avg = int(deg.sum()) // (C * NWIN)
    base_cap = max(WIN, (avg // WIN) * WIN)
    n_hi = max(1, (NWIN * 3) // 10)
    targets = np.array([base_cap + WIN] * n_hi + [base_cap] * (NWIN - n_hi),
                       np.int64)
    NEG = np.iinfo(np.int64).min
    for c in range(C):
        lo = c * S_NODES
        nodes = lo + np.argsort(-deg[lo:lo + S_NODES], kind="stable")
        counts = np.zeros(NWIN, np.int32)
        ld = loads[c]
        for n in nodes:
            room = targets - ld
            room[counts >= WIN] = NEG
            w = int(np.argmax(room))
            node_win[n] = w
            node_pos[n] = counts[w]
            counts[w] += 1
            ld[w] += deg[n]

    perm = np.full((C, SLICE), -1, np.int64)
    alln = np.arange(N)
    slot_global = node_win[alln] * WIN + node_pos[alln]
    perm[(alln // S_NODES), slot_global] = alln
    hrow = (alln // S_NODES) * SLICE + slot_global

    ew = node_win[dst]
    ebk = (hrow[src] // BANK).astype(np.int64)
    cnt = np.zeros((C, NWIN, NBK), np.int64)
    np.add.at(cnt, (core_of, ew, ebk), 1)
    K = np.maximum(1, -(-cnt.max(axis=0) // WIN))

    wb_base = np.zeros((NWIN, NBK), np.int64)
    off = 0
    half_meta = []
    for wlo in range(0, NWIN, 2):
        h0 = off
        runs = []
        for b in range(NBK):
            for w in (wlo, wlo + 1):
                wb_base[w, b] = off
                runs.append((b, w, int(K[w, b]), off))
                off += int(K[w, b]) * WIN
        half_meta.append((h0, off, runs))
    nslot = int(off)
    totch = nslot // WIN

    order = np.lexsort((np.arange(E), ebk, ew, core_of))
    src_s, dst_s, core_s = src[order], dst[order], core_of[order]
    grp_key = core_s * (NWIN * NBK) + ew[order] * NBK + ebk[order]
    uniq, start_idx, cnts = np.unique(grp_key, return_index=True,
                                      return_counts=True)
    run = np.arange(E) - np.repeat(start_idx, cnts)
    slot_of_edge = wb_base[ew[order], ebk[order]] + run

    src_img = np.zeros((C, totch * WIN), np.int32)
    dloc_img = np.full((C, totch * WIN), -1.0, np.float32)
    eperm = np.full((C, nslot), -1, np.int64)
    i16 = np.zeros((C, nslot), np.int16)
    src_img[core_s, slot_of_edge] = hrow[src_s].astype(np.int32)
    i16.reshape(C, -1)[core_s, slot_of_edge] = (hrow[src_s] % BANK).astype(np.int16)
    dloc_img[core_s, slot_of_edge] = node_pos[dst_s].astype(np.float32)
    eperm[core_s, slot_of_edge] = order

    idx32_img = np.ascontiguousarray(
        src_img.reshape(C, totch, WIN).transpose(0, 2, 1))
    dl_img = np.ascontiguousarray(
        dloc_img.reshape(C, totch, WIN).transpose(0, 2, 1))
    idx16_img = np.zeros((C, 128, nslot // 16), np.int16)
    sl = np.arange(nslot)
    for k in range(8):
        idx16_img[:, sl % 16 + 16 * k, sl // 16] = i16

    return dict(K=K, half_meta=half_meta, nslot=nslot, totch=totch,
                perm=perm, idx32_img=idx32_img, idx16_img=idx16_img,
                dl_img=dl_img, eperm=eperm, BANK=BANK)


def _build_inputs(inputs, pk):
    x = np.asarray(inputs["x"], np.float32)
    node_index = np.asarray(inputs["node_index"]).astype(np.int64)
    edge_attr = np.asarray(inputs["edge_attr"], np.float32)
    table = np.asarray(inputs["node_features_table"], np.float32)

    perm, eperm = pk["perm"], pk["eperm"]
    nslot = pk["nslot"]

    w_enc = np.asarray(inputs["W_enc"], np.float32)
    b_enc = np.asarray(inputs["b_enc"], np.float32)
    w_ohe = np.asarray(inputs["W_ohe"], np.float32)
    b_ohe = np.asarray(inputs["b_ohe"], np.float32)
    w_edge = np.asarray(inputs["W_edge"], np.float32)
    b_edge = np.asarray(inputs["b_edge"], np.float32)

    # nf row order on device: [u (x@W_ohe+b_ohe) rows 0-7; tg rows 8-15; ones]
    wenc_aug = np.concatenate([w_enc[8:16], w_enc[0:8], b_enc[None, :]], 0)
    wx_aug = np.concatenate([w_ohe, b_ohe[None, :]], 0)            # [9,8]
    wedge_aug = np.concatenate([w_edge, b_edge[None, :]], 0)       # [9,128]

    gcnw = np.asarray(inputs["gcn_W"], np.float32).reshape(-1, F)
    w1 = np.asarray(inputs["learner_W1"], np.float32).reshape(-1, HID)
    w2 = np.asarray(inputs["learner_W2"], np.float32).reshape(-1, F)
    wpred = np.asarray(inputs["W_pred"], np.float32)

    pb = np.zeros((F, 26), np.float32)
    pb[:, 0:6] = np.asarray(inputs["gcn_b"], np.float32).T
    pb[:, 6:12] = np.asarray(inputs["ln_gamma"], np.float32).T
    pb[:, 12:18] = np.asarray(inputs["ln_beta"], np.float32).T
    pb[:, 18:25] = -np.asarray(inputs["learner_b2"], np.float32).T
    pb[:TASKS, 25] = np.asarray(inputs["b_pred"], np.float32)
    b1s = np.ascontiguousarray(np.asarray(inputs["learner_b1"], np.float32).T)

    iota = np.broadcast_to(np.arange(128, dtype=np.float32)[None, :],
                           (128, 128)).astype(BF16)
    ident = np.eye(128, dtype=np.float32)
    rowc = np.zeros((1, 256), np.float32)
    rowc[0, :128] = 1.0
    rowc[0, 128:] = -1.0 / 128.0
    colc = np.zeros((128, 4), np.float32)
    colc[:, 0] = 1.0
    colc[:, 1] = 1e-16
    colc[:, 2] = LN_EPS

    maps = []
    for c in range(C):
        pm = perm[c]
        valid = pm >= 0
        xs = np.zeros((SLICE, 8), np.float32)
        xs[valid] = x[pm[valid]]
        tg = np.zeros((SLICE, 8), np.float32)
        tg[valid] = table[node_index[pm[valid]]]
        xT9 = np.zeros((9, SLICE), np.float32)
        xT9[:8] = xs.T
        xT9[8] = 1.0
        tgT = np.ascontiguousarray(tg.T)

        ep = eperm[c]
        ev = ep >= 0
        ea = np.zeros((nslot, 8), np.float32)
        ea[ev] = edge_attr[ep[ev]]
        attrT = np.zeros((9, nslot), np.float32)
        attrT[:8] = ea.T
        attrT[8, :] = 1.0

        maps.append({
            "idx16": pk["idx16_img"][c], "dloc": pk["dl_img"][c],
            "attrT": attrT, "xT9": xT9, "tgT": tgT,
            "iota": np.ascontiguousarray(iota), "ident": ident,
            "rowc": rowc, "colc": colc,
            "wenc": wenc_aug, "wx": wx_aug, "wedge": wedge_aug,
            "gcnw": gcnw, "w1": w1, "w2": w2, "wpred": wpred,
            "pb": pb, "b1s": b1s,
        })
    return maps


DEBUG_DUMP = False


# ---------------- bass program ----------------
def _build_program(pk, tvals):
    import concourse.bass as bass
    import concourse.tile as tile
    from concourse import bacc, mybir
    import concourse.hw_specs as hw_specs

    dt = mybir.dt
    AF = mybir.ActivationFunctionType
    OP = mybir.AluOpType

    class OneTableBacc(bacc.Bacc):
        """All activation funcs used here (Exp, Ln, Relu, Identity, Square)
        live in the natural_log_exp_and_others table set, but the stock
        chooser picks the first set containing each func (exp_and_others vs
        natural_log), thrashing ~2.7us ACT_TABLE_LOAD+DRAIN per switch.
        Strip Exp/Ln from every other set so one load serves the kernel."""

        def insert_act_table_loads(self):
            has_activation = any(
                isinstance(i, mybir.InstActivation)
                for b in self.main_func.blocks
                for i in b.instructions
            )
            if not has_activation:
                return
            tables = list(hw_specs.get_activation_tables(self.m.arch).items())
            assert any(n == "natural_log_exp_and_others" for n, _ in tables)
            filtered = [
                (n, f) if n == "natural_log_exp_and_others"
                else (n, f - {AF.Exp, AF.Ln})
                for n, f in tables
            ]
            bacc._bass_rust.insert_act_table_loads(self, filtered)

    K, half_meta = pk["K"], pk["half_meta"]
    nslot, totch, BANK = pk["nslot"], pk["totch"], pk["BANK"]
    HROWS = C * SLICE

    nc = OneTableBacc(num_devices=C, num_swdge_queues=4)

    t_idx16 = nc.dram_tensor("idx16", [128, nslot // 16], dt.int16,
                             kind="ExternalInput")
    t_dloc = nc.dram_tensor("dloc", [128, totch], dt.float32, kind="ExternalInput")
    t_attr = nc.dram_tensor("attrT", [9, nslot], dt.float32, kind="ExternalInput")
    t_x = nc.dram_tensor("xT9", [9, SLICE], dt.float32, kind="ExternalInput")
    t_tg = nc.dram_tensor("tgT", [8, SLICE], dt.float32, kind="ExternalInput")
    t_iota = nc.dram_tensor("iota", [128, 128], dt.bfloat16, kind="ExternalInput")
    t_ident = nc.dram_tensor("ident", [128, 128], dt.float32, kind="ExternalInput")
    t_rowc = nc.dram_tensor("rowc", [1, 256], dt.float32, kind="ExternalInput")
    t_colc = nc.dram_tensor("colc", [128, 4], dt.float32, kind="ExternalInput")
    t_wenc = nc.dram_tensor("wenc", [17, 128], dt.float32, kind="ExternalInput")
    t_wx = nc.dram_tensor("wx", [9, 8], dt.float32, kind="ExternalInput")
    t_wedge = nc.dram_tensor("wedge", [9, 128], dt.float32, kind="ExternalInput")
    t_gcnw = nc.dram_tensor("gcnw", [6 * F, F], dt.float32, kind="ExternalInput")
    t_w1 = nc.dram_tensor("w1", [7 * F, HID], dt.float32, kind="ExternalInput")
    t_w2 = nc.dram_tensor("w2", [7 * HID, F], dt.float32, kind="ExternalInput")
    t_wpred = nc.dram_tensor("wpred", [F, TASKS], dt.float32, kind="ExternalInput")
    t_pb = nc.dram_tensor("pb", [128, 26], dt.float32, kind="ExternalInput")
    t_b1 = nc.dram_tensor("b1s", [HID, 7], dt.float32, kind="ExternalInput")
    t_out = nc.dram_tensor("out", [SLICE, TASKS], dt.float32, kind="ExternalOutput")
    t_dbg = None
    if DEBUG_DUMP:
        t_dbg = nc.dram_tensor("dbg", [128, 2 * SLICE], dt.float32,
                               kind="ExternalOutput")


    h_full = nc.dram_tensor("h_full", [HROWS, F], dt.bfloat16,
                            addr_space="Shared")
    h_stage = nc.dram_tensor("h_stage", [SLICE, F], dt.bfloat16)
    emb_dram = nc.dram_tensor("emb_dram", [128, totch, 128], dt.bfloat16)

    rg = [list(range(C))]
    # edge phase runs per half-group (2 windows) to bound SBUF tiles
    hmax = max(h1 - h0 for (h0, h1, _) in half_meta)

    with tile.TileContext(nc) as tc:
        with (
            tc.tile_pool(name="const", bufs=1) as cp,
            tc.tile_pool(name="state", bufs=1) as sp,
            tc.tile_pool(name="work", bufs=8) as wp,
            tc.tile_pool(name="work2", bufs=2) as wp2,
            tc.tile_pool(name="edge", bufs=2) as epool,
            tc.tile_pool(name="edge1", bufs=1) as ep1,
            tc.tile_pool(name="psum", bufs=1, space="PSUM") as pp,
            tc.tile_pool(name="psum2", bufs=1, space="PSUM") as pp2,
        ):
            def load_const(tt, shape, dtype):
                s = cp.tile(shape, dtype, tag=tt.name, name=tt.name + "_sb")
                nc.sync.dma_start(out=s[:], in_=tt[:])
                return s

            iota_sb = load_const(t_iota, [128, 128], dt.bfloat16)
            ident_sb = load_const(t_ident, [128, 128], dt.float32)
            rowc_sb = load_const(t_rowc, [1, 256], dt.float32)
            colc_sb = load_const(t_colc, [128, 4], dt.float32)
            wenc_sb = load_const(t_wenc, [17, 128], dt.float32)
            wx_sb = load_const(t_wx, [9, 8], dt.float32)
            wedge_sb = load_const(t_wedge, [9, 128], dt.float32)
            wpred_sb = load_const(t_wpred, [128, TASKS], dt.float32)
            pb_sb = load_const(t_pb, [128, 26], dt.float32)
            b1_sb = load_const(t_b1, [HID, 7], dt.float32)
            i16_sb = load_const(t_idx16, [128, nslot // 16], dt.int16)
            dloc_sb = load_const(t_dloc, [128, totch], dt.float32)

            gcnw_sb = cp.tile([128, 6 * 128], dt.float32)
            w1_sb = cp.tile([128, 7 * HID], dt.float32)
            w2_sb = cp.tile([HID, 7 * 128], dt.float32)
            for l in range(6):
                nc.sync.dma_start(out=gcnw_sb[:, l * 128:(l + 1) * 128],
                                  in_=t_gcnw[l * 128:(l + 1) * 128, :])
            for l in range(7):
                nc.sync.dma_start(out=w1_sb[:, l * HID:(l + 1) * HID],
                                  in_=t_w1[l * F:(l + 1) * F, :])
                nc.sync.dma_start(out=w2_sb[:, l * 128:(l + 1) * 128],
                                  in_=t_w2[l * HID:(l + 1) * HID, :])

            hT = sp.tile([128, SLICE], dt.float32)
            cbT = sp.tile([128, SLICE], dt.float32)

            def t512(tag="t512"):
                pool = wp if tag == "t512" else wp2
                return pool.tile([128, 512], dt.float32, tag=tag, name=tag)

            def learner(lidx, zin):
                z1 = pp.tile([HID, 512], dt.float32, tag="zy")
                nc.tensor.matmul(z1[:], w1_sb[:, lidx * HID:(lidx + 1) * HID],
                                 zin[:])
                zb = t512()
                nc.vector.tensor_scalar(zb[:HID, :], z1[:],
                                        b1_sb[:, lidx:lidx + 1], None, OP.add)
                zs = t512()
                nc.vector.tensor_scalar(zs[:HID, :], zb[:HID, :], 0.2, None,
                                        OP.mult)
                z = t512()
                nc.vector.tensor_tensor(z[:HID, :], zb[:HID, :], zs[:HID, :],
                                        OP.max)
                y = pp.tile([128, 512], dt.float32, tag="zy")
                nc.tensor.matmul(y[:], w2_sb[:, lidx * 128:(lidx + 1) * 128],
                                 z[:HID, :])
                ee = t512()
                nc.scalar.activation(ee[:], y[:], AF.Exp,
                                     bias=pb_sb[:, 18 + lidx:19 + lidx], scale=-1.0)
                sps = t512()
                nc.scalar.activation(sps[:], ee[:], AF.Ln,
                                     bias=colc_sb[:, 0:1])
                nw = t512()
                nc.scalar.activation(nw[:], sps[:], AF.Exp, scale=-1.0)
                return nw

            def writeback(g, src_ap):
                stg = wp2.tile([128, 4, 128], dt.bfloat16, tag="stage")
                for w4 in range(4):
                    tr = pp.tile([128, 128], dt.float32, tag="zy")
                    nc.tensor.transpose(tr[:], src_ap[:, w4 * 128:(w4 + 1) * 128],
                                        ident_sb[:])
                    nc.vector.tensor_copy(stg[:, w4, :], tr[:])
                dst = h_stage[g * 512:(g + 1) * 512, :]
                dst = dst.rearrange("(w p) f -> p w f", p=128)
                nc.sync.dma_start(out=dst, in_=stg[:])

            # ---- prologue: h0, codebank, initial allgather ----
            for g in range(NGRP):
                cols = slice(g * 512, (g + 1) * 512)
                x9 = t512()
                nc.sync.dma_start(out=x9[:9, :], in_=t_x[:, cols])
                up = pp.tile([8, 512], dt.float32, tag="st")
                nc.tensor.matmul(up[:], wx_sb[:], x9[:9, :])
                nf = t512()
                nc.vector.tensor_copy(nf[0:8, :], up[:])
                nc.sync.dma_start(out=nf[8:16, :], in_=t_tg[:, cols])
                nc.sync.dma_start(out=nf[16:17, :], in_=t_x[8:9, cols])
                h0p = pp2.tile([128, 512], dt.float32, tag="h1")
                nc.tensor.matmul(h0p[:], wenc_sb[:], nf[:17, :])
                h0 = t512()
                nc.vector.tensor_copy(h0[:], h0p[:])
                nw = learner(0, h0)
                nc.vector.tensor_tensor(hT[:, cols], h0[:], nw[:], OP.mult)
                nc.vector.tensor_tensor(cbT[:, cols], hT[:, cols], nw[:], OP.mult)
                writeback(g, hT[:, cols])

            nc.gpsimd.collective_compute(
                "AllGather", OP.bypass, replica_groups=rg,
                ins=[h_stage[:, :]], outs=[h_full[:, :]])

            # ---- prologue: edge embeddings to DRAM (edge-major bf16) ----
            for j0 in range(0, totch, 8):
                j1 = min(j0 + 8, totch)
                a9 = ep1.tile([9, 8 * 128], dt.float32, tag="a9")
                nc.sync.dma_start(out=a9[:, :(j1 - j0) * 128],
                                  in_=t_attr[:, j0 * 128:j1 * 128])
                for jb in range(j0, j1, 4):
                    je = min(jb + 4, j1)
                    ep_ps = pp2.tile([128, 512], dt.float32, tag="h1")
                    for k in range(jb, je):
                        off = (k - j0) * 128
                        nc.tensor.matmul(
                            ep_ps[:, (k - jb) * 128:(k - jb + 1) * 128],
                            a9[:, off:off + 128], wedge_sb[:])
                    es = wp2.tile([128, 4, 128], dt.bfloat16, tag="stage")
                    nc.vector.tensor_copy(
                        es[:, :je - jb, :],
                        ep_ps[:, :(je - jb) * 128].rearrange(
                            "p (j f) -> p j f", f=128))
                    nc.sync.dma_start(out=emb_dram[:, jb:je, :],
                                      in_=es[:, :je - jb, :])

            # ---- layers ----
            for l in range(L):
                tl = tvals[l]
                for g in range(NGRP):
                    cols = slice(g * 512, (g + 1) * 512)

                    ndN = pp2.tile([128, 512], dt.float32, tag="num")
                    ndD = pp2.tile([128, 512], dt.float32, tag="den")
                    for half in range(2):
                        h0, h1, runs = half_meta[2 * g + half]
                        ne = h1 - h0

                        hs = epool.tile([128, hmax], dt.bfloat16, tag="eA")
                        j0 = h0 // 128
                        ncol = (h1 - h0) // 128
                        for b in range(NBK):
                            bruns = [r for r in runs if r[0] == b]
                            S0 = bruns[0][3]
                            S1 = bruns[-1][3] + bruns[-1][2] * 128
                            n = S1 - S0
                            nc.gpsimd.dma_gather(
                                out_ap=hs[:, S0 - h0:S1 - h0].rearrange(
                                    "p (j f) -> p j f", f=128),
                                in_ap=h_full[b * BANK:(b + 1) * BANK, :],
                                idxs_ap=i16_sb[:, S0 // 16:S1 // 16],
                                num_idxs=n,
                                num_idxs_reg=n,
                                elem_size=128,
                                queue_num=b,
                            )
                        if DEBUG_DUMP and l == 0 and g == 0 and half == 0:
                            hd = wp2.tile([128, hmax], dt.float32,
                                          tag="dbged", name="dbghs")
                            nc.vector.tensor_copy(hd[:, :ne], hs[:, :ne])
                            nc.sync.dma_start(out=t_dbg[:, :hmax], in_=hd[:])
                            hf_sb = wp2.tile([128, 512], dt.bfloat16,
                                             tag="dbghf", name="dbghf")
                            nc.sync.dma_start(
                                out=hf_sb[:].rearrange("p (a f) -> p a f", f=128),
                                in_=h_full[0:512, :].rearrange(
                                    "(a p) f -> p a f", p=128))
                            hf2 = wp2.tile([128, 512], dt.float32,
                                           tag="dbghf2", name="dbghf2")
                            nc.vector.tensor_copy(hf2[:], hf_sb[:])
                            nc.sync.dma_start(
                                out=t_dbg[:, SLICE:SLICE + 512], in_=hf2[:])
                        eb = epool.tile([128, hmax], dt.bfloat16, tag="eB")
                        nc.sync.dma_start(
                            out=eb[:, :ne].rearrange("p (j f) -> p j f", f=128),
                            in_=emb_dram[:, j0:j0 + ncol, :])
                        nc.vector.tensor_tensor(hs[:, :ne], hs[:, :ne],
                                                eb[:, :ne], OP.add)
                        nc.scalar.activation(eb[:, :ne], hs[:, :ne], AF.Relu)
                        ev = epool.tile([128, hmax], dt.bfloat16, tag="eC")
                        nc.scalar.activation(ev[:, :ne], eb[:, :ne], AF.Exp,
                                             scale=tl)
                        em = epool.tile([128, hmax], dt.bfloat16, tag="eD")
                        nc.vector.tensor_tensor(em[:, :ne], ev[:, :ne],
                                                eb[:, :ne], OP.mult)

                        oh = epool.tile([128, hmax], dt.bfloat16, tag="oh")
                        iap = iota_sb[:]
                        i3 = bass.AP(iap.tensor, iap.offset,
                                     [iap.ap[0], [0, ncol], iap.ap[1]])
                        dap = dloc_sb[:, j0:j0 + ncol]
                        d3 = bass.AP(dap.tensor, dap.offset,
                                     [dap.ap[0], dap.ap[1], [0, 128]])
                        nc.vector.tensor_tensor(
                            oh[:, :ne].rearrange("p (j f) -> p j f", f=128),
                            i3, d3, OP.is_equal)
                        # chunk matmuls; window w accumulation spans its
                        # per-bank runs (start at first, stop at last)
                        for (b, w, kw, S0) in sorted(runs,
                                                     key=lambda r: (r[1], r[0])):
                            w4 = w - 4 * g
                            for k in range(kw):
                                off = S0 - h0 + k * 128
                                st = (b == 0 and k == 0)
                                sp = (b == NBK - 1 and k == kw - 1)
                                nc.tensor.matmul(
                                    ndN[:, w4 * 128:(w4 + 1) * 128],
                                    em[:, off:off + 128],
                                    oh[:, off:off + 128],
                                    start=st, stop=sp,
                                    skip_group_check=True)
                                nc.tensor.matmul(
                                    ndD[:, w4 * 128:(w4 + 1) * 128],
                                    ev[:, off:off + 128],
                                    oh[:, off:off + 128],
                                    start=st, stop=sp,
                                    skip_group_check=True)

                    lnd = t512()
                    nc.scalar.activation(lnd[:], ndD[:], AF.Ln,
                                         bias=colc_sb[:, 1:2])
                    rec = t512()
                    nc.scalar.activation(rec[:], lnd[:], AF.Exp, scale=-1.0)
                    hh = t512()
                    nc.vector.tensor_tensor(hh[:], ndN[:], rec[:], OP.mult)
                    nc.vector.tensor_tensor(hh[:], hh[:], hT[:, cols], OP.add)
                    if False:
                        nd_sb = t512("dbgnum")
                        nc.vector.tensor_copy(nd_sb[:], ndN[:])
                        nc.sync.dma_start(out=t_dbg[:, g * 512:(g + 1) * 512],
                                          in_=nd_sb[:])
                        dd_sb = t512("dbgnum")
                        nc.vector.tensor_copy(dd_sb[:], ndD[:])
                        nc.sync.dma_start(
                            out=t_dbg[:, SLICE + g * 512:SLICE + (g + 1) * 512],
                            in_=dd_sb[:])

                    h1p = pp2.tile([128, 512], dt.float32, tag="h1")
                    nc.tensor.matmul(h1p[:], gcnw_sb[:, l * 128:(l + 1) * 128],
                                     hh[:])
                    h1 = t512()
                    nc.scalar.activation(h1[:], h1p[:], AF.Identity,
                                         bias=pb_sb[:, l:l + 1])
                    sq = t512()
                    nc.scalar.activation(sq[:], h1[:], AF.Square)
                    sts = pp.tile([1, 512], dt.float32, tag="st")
                    stq = pp.tile([1, 512], dt.float32, tag="st2")
                    nc.tensor.matmul(sts[:], colc_sb[:, 0:1], h1[:])
                    nc.tensor.matmul(stq[:], colc_sb[:, 0:1], sq[:])
                    m2 = t512()
                    nc.scalar.activation(m2[:1, :], sts[:], AF.Square,
                                         scale=float(1.0 / np.sqrt(128.0)))
                    dv = t512()
                    nc.vector.tensor_tensor(dv[:1, :], stq[:], m2[:1, :],
                                            OP.subtract)
                    lnv = t512()
                    nc.scalar.activation(lnv[:1, :], dv[:1, :], AF.Ln,
                                         bias=colc_sb[:1, 2:3],
                                         scale=float(1.0 / 128.0))
                    rst = t512()
                    nc.scalar.activation(rst[:1, :], lnv[:1, :], AF.Exp,
                                         scale=-0.5)
                    tmu = t512()
                    nc.vector.tensor_tensor(tmu[:1, :], sts[:], rst[:1, :],
                                            OP.mult)
                    aB = pp.tile([128, 512], dt.float32, tag="ab")
                    cB = pp.tile([128, 512], dt.float32, tag="cb")
                    nc.tensor.matmul(aB[:], rowc_sb[:, 0:128], rst[:1, :])
                    nc.tensor.matmul(cB[:], rowc_sb[:, 128:256], tmu[:1, :])
                    hn = t512("hn")
                    nc.vector.tensor_tensor(hn[:], h1[:], aB[:], OP.mult)
                    nc.vector.tensor_tensor(hn[:], hn[:], cB[:], OP.add)
                    nc.vector.tensor_scalar(hn[:], hn[:], pb_sb[:, 6 + l:7 + l],
                                            pb_sb[:, 12 + l:13 + l],
                                            OP.mult, OP.add)
                    nc.vector.tensor_scalar(hn[:], hn[:], 0.0, None, OP.max)

                    zin = t512()
                    nc.vector.tensor_tensor(zin[:], hn[:], cbT[:, cols], OP.add)
                    nw = learner(l + 1, zin)
                    hf = t512("hf")
                    nc.vector.tensor_tensor(hf[:], hn[:], nw[:], OP.mult)
                    qq = t512("qq")
                    nc.vector.tensor_tensor(qq[:], cbT[:, cols], nw[:], OP.mult)
                    nc.vector.tensor_tensor(cbT[:, cols], cbT[:, cols], hf[:],
                                            OP.add)
                    nc.vector.tensor_tensor(hT[:, cols], cbT[:, cols], qq[:],
                                            OP.subtract)
                    writeback(g, hT[:, cols])

                nc.gpsimd.collective_compute(
                    "AllGather", OP.bypass, replica_groups=rg,
                    ins=[h_stage[:, :]], outs=[h_full[:, :]])

            # ---- epilogue ----
            for g in range(NGRP):
                cols = slice(g * 512, (g + 1) * 512)
                op_ps = pp2.tile([TASKS, 512], dt.float32, tag="h1")
                nc.tensor.matmul(op_ps[:], wpred_sb[:], cbT[:, cols])
                ot = t512("ot")
                nc.vector.tensor_scalar(ot[:TASKS, :], op_ps[:],
                                        pb_sb[:TASKS, 25:26], None, OP.add)
                for w4 in range(4):
                    tr = pp.tile([128, TASKS], dt.float32, tag="zy")
                    nc.tensor.transpose(tr[:], ot[:TASKS,
                                                  w4 * 128:(w4 + 1) * 128],
                                        ident_sb[:TASKS, :TASKS])
                    os_ = t512("ot")
                    nc.vector.tensor_copy(os_[:, :TASKS], tr[:])
                    r0 = g * 512 + w4 * 128
                    nc.sync.dma_start(out=t_out[r0:r0 + 128, :],
                                      in_=os_[:, :TASKS])

    return nc


# ---------------- entry point ----------------
def kernel(**inputs):
    from concourse.bass_utils import run_bass_kernel_spmd

    pk = _pack_graph(np.asarray(inputs["edge_index"]))
    maps = _build_inputs(inputs, pk)
    tvals = [float(v) for v in np.asarray(inputs["gcn_t"], np.float32)]

    nc = _build_program(pk, tvals)
    if not nc.is_finalized():
        nc.finalize()
    trace = bool(int(os.environ.get("KERNEL_PROFILE", "0")))
    res = run_bass_kernel_spmd(nc, maps, list(range(C)), trace=trace)
    kernel.exec_time_ns = res.exec_time_ns
    kernel.profile_json = res.profile_json

    out = np.zeros((N, TASKS), np.float32)
    for c in range(C):
        oc = np.asarray(res.results[c]["out"], np.float32)
        pm = pk["perm"][c]
        valid = pm >= 0
        out[pm[valid]] = oc[valid]
    if DEBUG_DUMP:
        kernel.dbg = [np.asarray(res.results[c].get("dbg")) for c in range(C)]
        kernel.pk = pk
    return out

